# revision 3
# baseline (speedup 1.0000x reference)
"""Trainium2 Bass kernel for nn_Attention_78048145703090 (sparse_attention).

Math: the reference's [N,N] attention is rank-1 structured. Every row n of the
logit matrix is w_n * s where s[m] = scale * (q_center . k_m) is one shared
score vector per sample and w_n = exp(1 - dist_n) > 0 depends only on the grid
distance of n from the center. Softmax rows therefore only depend on w_n, and
only U=457 distinct w_n values exist on the 64x64 grid. The kernel computes
the 457 unique softmax rows (exp + matmul against V), projects them, and
expands back to 4096 rows with a one-hot gather matmul.

Additional contraction: s = xf @ (scale * wk^T q_c) + scale*(q_c . bk), so K is
never materialized; s is fused as a 65th output column of the V projection.

Sharding: data-parallel over B=8 across the 8 cores (one sample per core);
each core holds the full 64x64 weights.
"""

import sys

sys.path.insert(0, "/opt/trn_rl_repo")

import numpy as np

import concourse.bacc as bacc
import concourse.mybir as mybir
import concourse.tile as tile
from concourse import masks


def _install_profile_hook():
    """This image's antenv lacks axon_hooks; reconstruct it so
    run_bass_kernel_spmd(trace=True) can capture NTFF profiles. No-op for
    normal (untraced) runs."""
    import types

    try:
        import antenv.axon_hooks  # noqa: F401

        return
    except ImportError:
        pass
    try:
        import antenv

        m = types.ModuleType("antenv.axon_hooks")
        state = {"hook": None}
        m.set_axon_ntff_profile_hook = lambda h: state.__setitem__("hook", h)
        m.get_axon_ntff_profile_hook = lambda: state["hook"]
        sys.modules["antenv.axon_hooks"] = m
        antenv.axon_hooks = m
        from trn_agent_boot.trn_boot import _ntff_profile_via_ctypes

        m.set_axon_ntff_profile_hook(
            _ntff_profile_via_ctypes("/opt/axon/libaxon_pjrt.so")
        )
    except Exception:
        pass


_install_profile_hook()

from concourse.bass_utils import run_bass_kernel_spmd

B, H, W, C = 8, 64, 64, 64
N = H * W  # 4096
P = 128
NCH = N // P  # 32
CENTER = (H // 2) * W + (W // 2)  # 2080
SCALE = float(C) ** -0.5
F32 = mybir.dt.float32

# ---- compile-time constants derived from the distance grid ----
_yy, _xx = np.mgrid[0:H, 0:W]
_d2 = ((_yy - H // 2) ** 2 + (_xx - W // 2) ** 2).reshape(-1)  # [N] int
_uniq_d2, _g = np.unique(_d2, return_inverse=True)
U = len(_uniq_d2)  # 457
UP = 512  # padded to 4 partition chunks
JC = UP // P  # 4
W_U = np.zeros((1, UP), np.float32)
W_U[0, :U] = np.exp(np.float32(1.0) - np.sqrt(_uniq_d2.astype(np.float32)))
# one-hot gather matrix, packed [P, JC, N]: gt[p, jc, n] = (g[n] == jc*P + p)
GT = np.zeros((P, JC, N), np.float32)
GT[_g % P, _g // P, np.arange(N)] = 1.0


def build_nc():
    nc = bacc.Bacc("TRN2", target_bir_lowering=False, debug=False, num_devices=B)
    xb = nc.dram_tensor("xb", [N, C], F32, kind="ExternalInput")
    wq1 = nc.dram_tensor("wq1", [C + 1, C], F32, kind="ExternalInput")
    wkn = nc.dram_tensor("wkn", [C, C], F32, kind="ExternalInput")
    bkc = nc.dram_tensor("bkc", [C, 1], F32, kind="ExternalInput")
    wv1 = nc.dram_tensor("wv1", [C + 1, C], F32, kind="ExternalInput")
    wp1 = nc.dram_tensor("wp1", [C + 1, C], F32, kind="ExternalInput")
    wu = nc.dram_tensor("wu", [1, UP], F32, kind="ExternalInput")
    gt = nc.dram_tensor("gt", [P, JC, N], F32, kind="ExternalInput")
    out = nc.dram_tensor("out", [N, C], F32, kind="ExternalOutput")

    with tile.TileContext(nc) as tc:
        with (
            tc.tile_pool(name="consts", bufs=1) as consts,
            tc.tile_pool(name="sb", bufs=1) as sb,
            tc.tile_pool(name="epool", bufs=4) as epool,
            tc.tile_pool(name="opool", bufs=3) as opool,
            tc.tile_pool(name="ps_t", bufs=2, space="PSUM") as ps_t,
            tc.tile_pool(name="ps_vs", bufs=2, space="PSUM") as ps_vs,
            tc.tile_pool(name="ps_nd", bufs=1, space="PSUM") as ps_nd,
            tc.tile_pool(name="ps_small", bufs=2, space="PSUM") as ps_small,
        ):
            ident = consts.tile([P, P], F32)
            masks.make_identity(nc, ident[:])
            ones_row = consts.tile([1, P], F32)
            nc.vector.memset(ones_row[:], 1.0)

            gt_sb = consts.tile([P, JC, N], F32)
            nc.sync.dma_start(out=gt_sb[:], in_=gt[:])
            wq1_sb = consts.tile([C + 1, C], F32)
            nc.sync.dma_start(out=wq1_sb[:], in_=wq1[:])
            wkn_sb = consts.tile([C, C], F32)
            nc.sync.dma_start(out=wkn_sb[:], in_=wkn[:])
            bkc_sb = consts.tile([C, 1], F32)
            nc.sync.dma_start(out=bkc_sb[:], in_=bkc[:])
            wp1_sb = consts.tile([C + 1, C], F32)
            nc.sync.dma_start(out=wp1_sb[:], in_=wp1[:])
            wu_sb = consts.tile([1, UP], F32)
            nc.sync.dma_start(out=wu_sb[:], in_=wu[:])
            wvu_sb = consts.tile([C + 1, C + 1], F32)
            nc.sync.dma_start(out=wvu_sb[:, 0 : C], in_=wv1[:])

            x_sb = sb.tile([P, NCH, C], F32)
            nc.sync.dma_start(
                out=x_sb[:], in_=xb.ap().rearrange("(i p) c -> p i c", p=P)
            )

            # xfT1 [65, N]: transposed sample with a ones row for bias folding
            xfT1 = sb.tile([C + 1, N], F32)
            nc.vector.memset(xfT1[C : C + 1, :], 1.0)
            for i in range(NCH):
                tp = ps_t.tile([C, P], F32, tag="t")
                nc.tensor.transpose(out=tp[:], in_=x_sb[:, i, :], identity=ident[:])
                nc.scalar.copy(out=xfT1[0:C, i * P : (i + 1) * P], in_=tp[:])

            # q_center, then u = wk^T q_c and c0 = q_c . bk (scaled later)
            qc_ps = ps_small.tile([C, 1], F32, tag="m")
            nc.tensor.matmul(
                qc_ps[:], wq1_sb[:], xfT1[:, CENTER : CENTER + 1], start=True, stop=True
            )
            qc_sb = sb.tile([C, 1], F32)
            nc.vector.tensor_copy(out=qc_sb[:], in_=qc_ps[:])
            u_ps = ps_small.tile([C, 1], F32, tag="m")
            nc.tensor.matmul(u_ps[:], wkn_sb[:], qc_sb[:], start=True, stop=True)
            c0_ps = ps_small.tile([1, 1], F32, tag="m")
            nc.tensor.matmul(c0_ps[:], bkc_sb[:], qc_sb[:], start=True, stop=True)
            nc.scalar.mul(out=wvu_sb[0:C, C : C + 1], in_=u_ps[:], mul=SCALE)
            nc.scalar.mul(out=wvu_sb[C : C + 1, C : C + 1], in_=c0_ps[:], mul=SCALE)

            # V projection fused with scores: out cols 0..63 = v row, col 64 = s
            v1_sb = sb.tile([P, NCH, C + 1], F32)
            nc.vector.memset(v1_sb[:, :, C : C + 1], 1.0)
            s_col = sb.tile([P, NCH], F32)
            for i in range(NCH):
                vs = ps_vs.tile([P, C + 1], F32)
                nc.tensor.matmul(
                    vs[:], xfT1[:, i * P : (i + 1) * P], wvu_sb[:], start=True, stop=True
                )
                nc.vector.tensor_copy(out=v1_sb[:, i, 0:C], in_=vs[:, 0:C])
                nc.vector.tensor_copy(out=s_col[:, i : i + 1], in_=vs[:, C : C + 1])

            # global max of s (w_n > 0 so row max = w_n * max(s))
            mx = sb.tile([P, 1], F32)
            nc.vector.reduce_max(out=mx[:], in_=s_col[:], axis=mybir.AxisListType.X)
            mxT = ps_small.tile([1, P], F32, tag="m")
            nc.tensor.transpose(out=mxT[:], in_=mx[:], identity=ident[:])
            mxT_sb = sb.tile([1, P], F32)
            nc.vector.tensor_copy(out=mxT_sb[:], in_=mxT[:])
            mg = sb.tile([1, 1], F32)
            nc.vector.reduce_max(out=mg[:], in_=mxT_sb[:], axis=mybir.AxisListType.X)
            mb_ps = ps_small.tile([P, 1], F32, tag="m")
            nc.tensor.matmul(mb_ps[:], ones_row[:], mg[:], start=True, stop=True)
            mb_sb = sb.tile([P, 1], F32)
            nc.vector.tensor_copy(out=mb_sb[:], in_=mb_ps[:])
            shm = sb.tile([P, NCH], F32)
            nc.vector.tensor_scalar_sub(shm[:], s_col[:], mb_sb[:])

            # unique weights broadcast across partitions
            wb_ps = ps_small.tile([P, UP], F32, tag="m")
            nc.tensor.matmul(wb_ps[:], ones_row[:], wu_sb[:], start=True, stop=True)
            wb_sb = sb.tile([P, UP], F32)
            nc.vector.tensor_copy(out=wb_sb[:], in_=wb_ps[:])

            # E'[m, j] = exp(sh[m] * w_u[j]) per chunk; accumulate [V|1]^T E'
            nd_ps = ps_nd.tile([C + 1, UP], F32)
            for i in range(NCH):
                e_i = epool.tile([P, UP], F32)
                nc.scalar.activation(
                    out=e_i[:],
                    in_=wb_sb[:],
                    func=mybir.ActivationFunctionType.Exp,
                    scale=shm[:, i : i + 1],
                )
                nc.tensor.matmul(
                    nd_ps[:],
                    v1_sb[:, i, :],
                    e_i[:],
                    start=(i == 0),
                    stop=(i == NCH - 1),
                )

            # o^T = num^T * (1/den) broadcast, with ones row for bias folding
            r_sb = sb.tile([1, UP], F32)
            nc.vector.reciprocal(out=r_sb[:], in_=nd_ps[C : C + 1, :])
            rb_ps = ps_small.tile([C, UP], F32, tag="m")
            nc.tensor.matmul(rb_ps[:], ones_row[:, 0:C], r_sb[:], start=True, stop=True)
            rb_sb = sb.tile([C, UP], F32)
            nc.vector.tensor_copy(out=rb_sb[:], in_=rb_ps[:])
            oT1 = sb.tile([C + 1, UP], F32)
            nc.vector.memset(oT1[C : C + 1, :], 1.0)
            nc.vector.tensor_mul(oT1[0:C, :], nd_ps[0:C, :], rb_sb[:])

            # p^T = [wp.T|bp]^T @ oT1 -> [C, UP], then transpose to p chunks
            pT_ps = ps_small.tile([C, UP], F32, tag="m")
            nc.tensor.matmul(pT_ps[:], wp1_sb[:], oT1[:], start=True, stop=True)
            pT_sb = sb.tile([C, UP], F32)
            nc.vector.tensor_copy(out=pT_sb[:], in_=pT_ps[:])
            p_sb = sb.tile([P, JC, C], F32)
            for jc in range(JC):
                tp2 = ps_t.tile([P, C], F32, tag="t")
                nc.tensor.transpose(
                    out=tp2[:],
                    in_=pT_sb[:, jc * P : (jc + 1) * P],
                    identity=ident[0:C, 0:C],
                )
                nc.vector.tensor_copy(out=p_sb[:, jc, :], in_=tp2[:])

            # expand unique rows to all 4096 positions: out = G @ p
            for t in range(NCH):
                og = ps_t.tile([P, C], F32, tag="t")
                for jc in range(JC):
                    nc.tensor.matmul(
                        og[:],
                        gt_sb[:, jc, t * P : (t + 1) * P],
                        p_sb[:, jc, :],
                        start=(jc == 0),
                        stop=(jc == JC - 1),
                    )
                o_sb = opool.tile([P, C], F32)
                nc.vector.tensor_copy(out=o_sb[:], in_=og[:])
                nc.sync.dma_start(out=out[t * P : (t + 1) * P, :], in_=o_sb[:])

    nc.compile()
    return nc


_nc_cache = None


def _get_nc():
    global _nc_cache
    if _nc_cache is None:
        _nc_cache = build_nc()
    return _nc_cache


def make_in_maps(x, wq, bq, wk, bk, wv, bv, wp, bp):
    f = lambda a: np.ascontiguousarray(np.asarray(a, dtype=np.float32))
    x = f(x)
    shared = {
        "wq1": np.concatenate([f(wq).T, f(bq)[None, :]], 0),
        "wkn": f(wk),
        "bkc": f(bk)[:, None],
        "wv1": np.concatenate([f(wv).T, f(bv)[None, :]], 0),
        "wp1": np.concatenate([f(wp).T, f(bp)[None, :]], 0),
        "wu": W_U,
        "gt": GT,
    }
    shared = {k: np.ascontiguousarray(v) for k, v in shared.items()}
    return [
        {"xb": np.ascontiguousarray(x[b].reshape(N, C)), **shared} for b in range(B)
    ]


def kernel_with_results(trace=False, **inputs):
    in_maps = make_in_maps(**inputs)
    nc = _get_nc()
    res = run_bass_kernel_spmd(nc, in_maps, core_ids=list(range(B)), trace=trace)
    out = np.stack([r["out"] for r in res.results], 0).reshape(B, H, W, C)
    return out, res


def kernel(**inputs):
    out, _ = kernel_with_results(**inputs)
    return out


# revision 5
# speedup vs baseline: 1.3110x; 1.3110x over previous
"""Trainium2 Bass kernel for nn_Attention_78048145703090 (sparse_attention).

Math: the reference's [N,N] attention is rank-1 structured. Every row n of the
logit matrix is w_n * s where s[m] = scale * (q_center . k_m) is one shared
score vector per sample and w_n = exp(1 - dist_n) > 0 depends only on the grid
distance of n from the center. Softmax rows therefore only depend on w_n, and
only U=457 distinct w_n values exist on the 64x64 grid. The kernel computes
the 457 unique softmax rows, projects them, and expands back to 4096 rows
with a one-hot gather matmul.

Contractions used:
  - s = xf @ (scale * wk^T q_c) (+ const): row-constant terms drop out of
    softmax, so bk never enters; s is computed by a DVE mul+reduce against
    x in natural layout.
  - num = E' @ V = (E' @ xf) @ wv^T + den * bv, so V is never materialized
    and x is consumed in natural [m, c] layout as the matmul stationary
    operand (no input transposes at all).

Sharding: data-parallel over B=8 across the 8 cores (one sample per core);
each core holds the full 64x64 weights.
"""

import sys

sys.path.insert(0, "/opt/trn_rl_repo")

import numpy as np

import concourse.bacc as bacc
import concourse.mybir as mybir
import concourse.tile as tile
from concourse import masks


def _install_profile_hook():
    """This image's antenv lacks axon_hooks; reconstruct it so
    run_bass_kernel_spmd(trace=True) can capture NTFF profiles. No-op for
    normal (untraced) runs."""
    import types

    try:
        import antenv.axon_hooks  # noqa: F401

        return
    except ImportError:
        pass
    try:
        import antenv

        m = types.ModuleType("antenv.axon_hooks")
        state = {"hook": None}
        m.set_axon_ntff_profile_hook = lambda h: state.__setitem__("hook", h)
        m.get_axon_ntff_profile_hook = lambda: state["hook"]
        sys.modules["antenv.axon_hooks"] = m
        antenv.axon_hooks = m
        from trn_agent_boot.trn_boot import _ntff_profile_via_ctypes

        m.set_axon_ntff_profile_hook(
            _ntff_profile_via_ctypes("/opt/axon/libaxon_pjrt.so")
        )
    except Exception:
        pass


_install_profile_hook()

from concourse.bass_utils import run_bass_kernel_spmd

B, H, W, C = 8, 64, 64, 64
N = H * W  # 4096
P = 128
NCH = N // P  # 32
CENTER = (H // 2) * W + (W // 2)  # 2080
C_CH = CENTER // P  # chunk holding the center row
C_PT = CENTER % P  # partition holding the center row
SCALE = float(C) ** -0.5
F32 = mybir.dt.float32
NS = 8  # output column slices for the gather (N / 512)

# ---- compile-time constants derived from the distance grid ----
_yy, _xx = np.mgrid[0:H, 0:W]
_d2 = ((_yy - H // 2) ** 2 + (_xx - W // 2) ** 2).reshape(-1)  # [N] int
_uniq_d2, _g = np.unique(_d2, return_inverse=True)
U = len(_uniq_d2)  # 457
UP = 512  # padded to 4 partition chunks
JC = UP // P  # 4
W_U = np.zeros((1, UP), np.float32)
W_U[0, :U] = np.exp(np.float32(1.0) - np.sqrt(_uniq_d2.astype(np.float32)))
# one-hot gather matrix, packed [P, JC, N]: gt[p, jc, n] = (g[n] == jc*P + p)
GT = np.zeros((P, JC, N), np.float32)
GT[_g % P, _g // P, np.arange(N)] = 1.0


def build_nc():
    nc = bacc.Bacc("TRN2", target_bir_lowering=False, debug=False, num_devices=B)
    xb = nc.dram_tensor("xb", [N, C], F32, kind="ExternalInput")
    wq1 = nc.dram_tensor("wq1", [C + 1, C], F32, kind="ExternalInput")
    wkn = nc.dram_tensor("wkn", [C, C], F32, kind="ExternalInput")
    wv1 = nc.dram_tensor("wv1", [C + 1, C], F32, kind="ExternalInput")
    wp1 = nc.dram_tensor("wp1", [C + 1, C], F32, kind="ExternalInput")
    wu = nc.dram_tensor("wu", [1, UP], F32, kind="ExternalInput")
    gt = nc.dram_tensor("gt", [P, JC, N], F32, kind="ExternalInput")
    out = nc.dram_tensor("out", [N, C], F32, kind="ExternalOutput")

    with tile.TileContext(nc) as tc:
        with (
            tc.tile_pool(name="consts", bufs=1) as consts,
            tc.tile_pool(name="sb", bufs=1) as sb,
            tc.tile_pool(name="xu_pool", bufs=2) as xu_pool,
            tc.tile_pool(name="epool", bufs=4) as epool,
            tc.tile_pool(name="opool", bufs=4) as opool,
            tc.tile_pool(name="obt_sb_pool", bufs=2) as obt_sb_pool,
            tc.tile_pool(name="ps_t", bufs=2, space="PSUM") as ps_t,
            tc.tile_pool(name="ps_yt", bufs=1, space="PSUM") as ps_yt,
            tc.tile_pool(name="ps_small", bufs=2, space="PSUM") as ps_small,
            tc.tile_pool(name="ps_ob", bufs=3, space="PSUM") as ps_ob,
        ):
            ident = consts.tile([P, P], F32)
            masks.make_identity(nc, ident[:])
            ones_row = consts.tile([1, P], F32)
            nc.vector.memset(ones_row[:], 1.0)

            gt_sb = consts.tile([P, JC, N], F32)
            nc.sync.dma_start(out=gt_sb[:], in_=gt[:])
            wq1_sb = consts.tile([C + 1, C], F32)
            nc.sync.dma_start(out=wq1_sb[:], in_=wq1[:])
            wkn_sb = consts.tile([C, C], F32)
            nc.sync.dma_start(out=wkn_sb[:], in_=wkn[:])
            wv1_sb = consts.tile([C + 1, C], F32)
            nc.sync.dma_start(out=wv1_sb[:], in_=wv1[:])
            wp1_sb = consts.tile([C + 1, C], F32)
            nc.sync.dma_start(out=wp1_sb[:], in_=wp1[:])
            wu_sb = consts.tile([1, UP], F32)
            nc.sync.dma_start(out=wu_sb[:], in_=wu[:])

            # x in natural layout with a ones column for bias/den folding
            x1_sb = sb.tile([P, NCH, C + 1], F32)
            nc.vector.memset(x1_sb[:, :, C : C + 1], 1.0)
            nc.sync.dma_start(
                out=x1_sb[:, :, 0:C],
                in_=xb.ap().rearrange("(i p) c -> p i c", p=P),
            )

            # q_center: transpose the center row to a column, project
            qcr_sb = sb.tile([C + 1, 1], F32)
            nc.vector.memset(qcr_sb[:], 1.0)
            xrow_ps = ps_small.tile([C, P], F32, tag="m")
            nc.tensor.transpose(
                out=xrow_ps[:],
                in_=x1_sb[:, C_CH, 0:C],
                identity=ident[:],
            )
            nc.vector.tensor_copy(out=qcr_sb[0:C, :], in_=xrow_ps[:, C_PT : C_PT + 1])
            qc_ps = ps_small.tile([C, 1], F32, tag="m")
            nc.tensor.matmul(qc_ps[:], wq1_sb[:], qcr_sb[:], start=True, stop=True)
            qc_sb = sb.tile([C, 1], F32)
            nc.vector.tensor_copy(out=qc_sb[:], in_=qc_ps[:])

            # u_row = scale * (q_c^T wk) as a row, broadcast across partitions
            ur_ps = ps_small.tile([1, C], F32, tag="m")
            nc.tensor.matmul(ur_ps[:], qc_sb[:], wkn_sb[:], start=True, stop=True)
            ur_sb = sb.tile([1, C], F32)
            nc.scalar.mul(out=ur_sb[:], in_=ur_ps[:], mul=SCALE)
            ubc_ps = ps_small.tile([P, C], F32, tag="m")
            nc.tensor.matmul(ubc_ps[:], ones_row[:], ur_sb[:], start=True, stop=True)
            ubc_sb = sb.tile([P, C], F32)
            nc.vector.tensor_copy(out=ubc_sb[:], in_=ubc_ps[:])

            # s[m] = x[m, :] . u  via DVE mul + reduce per chunk
            s_col = sb.tile([P, NCH], F32)
            for i in range(NCH):
                xu = xu_pool.tile([P, C], F32)
                nc.vector.tensor_mul(xu[:], x1_sb[:, i, 0:C], ubc_sb[:])
                nc.vector.reduce_sum(
                    out=s_col[:, i : i + 1], in_=xu[:], axis=mybir.AxisListType.X
                )

            # global max of s (w_n > 0 so row max = w_n * max(s))
            mx = sb.tile([P, 1], F32)
            nc.vector.reduce_max(out=mx[:], in_=s_col[:], axis=mybir.AxisListType.X)
            mxT = ps_small.tile([1, P], F32, tag="m")
            nc.tensor.transpose(out=mxT[:], in_=mx[:], identity=ident[:])
            mxT_sb = sb.tile([1, P], F32)
            nc.vector.tensor_copy(out=mxT_sb[:], in_=mxT[:])
            mg = sb.tile([1, 1], F32)
            nc.vector.reduce_max(out=mg[:], in_=mxT_sb[:], axis=mybir.AxisListType.X)
            mb_ps = ps_small.tile([P, 1], F32, tag="m")
            nc.tensor.matmul(mb_ps[:], ones_row[:], mg[:], start=True, stop=True)
            mb_sb = sb.tile([P, 1], F32)
            nc.vector.tensor_copy(out=mb_sb[:], in_=mb_ps[:])
            shm = sb.tile([P, NCH], F32)
            nc.vector.tensor_scalar_sub(shm[:], s_col[:], mb_sb[:])

            # unique weights broadcast across partitions
            wb_ps = ps_small.tile([P, UP], F32, tag="m")
            nc.tensor.matmul(wb_ps[:], ones_row[:], wu_sb[:], start=True, stop=True)
            wb_sb = sb.tile([P, UP], F32)
            nc.vector.tensor_copy(out=wb_sb[:], in_=wb_ps[:])

            # E'[m, j] = exp(sh[m] * w_u[j]); accumulate YT = [x|1]^T E'
            # rows 0..63 = (E' @ xf)^T, row 64 = den
            yt_ps = ps_yt.tile([C + 1, UP], F32)
            for i in range(NCH):
                e_i = epool.tile([P, UP], F32)
                nc.scalar.activation(
                    out=e_i[:],
                    in_=wb_sb[:],
                    func=mybir.ActivationFunctionType.Exp,
                    scale=shm[:, i : i + 1],
                )
                nc.tensor.matmul(
                    yt_ps[:],
                    x1_sb[:, i, :],
                    e_i[:],
                    start=(i == 0),
                    stop=(i == NCH - 1),
                )

            ytd_sb = sb.tile([C + 1, UP], F32)
            nc.vector.tensor_copy(out=ytd_sb[:], in_=yt_ps[:])
            # num^T = [wv.T|bv]^T @ [Y|den]  (bias folds against the den row)
            numT_ps = ps_small.tile([C, UP], F32, tag="m")
            nc.tensor.matmul(numT_ps[:], wv1_sb[:], ytd_sb[:], start=True, stop=True)
            # r = 1/den broadcast across partitions
            r_sb = sb.tile([1, UP], F32)
            nc.vector.reciprocal(out=r_sb[:], in_=ytd_sb[C : C + 1, :])
            rb_ps = ps_small.tile([C, UP], F32, tag="m")
            nc.tensor.matmul(rb_ps[:], ones_row[:, 0:C], r_sb[:], start=True, stop=True)
            rb_sb = sb.tile([C, UP], F32)
            nc.vector.tensor_copy(out=rb_sb[:], in_=rb_ps[:])
            # o^T = num^T * r, with a ones row for the bias fold of wp
            oT1 = sb.tile([C + 1, UP], F32)
            nc.vector.memset(oT1[C : C + 1, :], 1.0)
            nc.vector.tensor_mul(oT1[0:C, :], numT_ps[:], rb_sb[:])

            # p^T = [wp.T|bp]^T @ oT1 -> [C, UP], then transpose to p chunks
            pT_ps = ps_small.tile([C, UP], F32, tag="m")
            nc.tensor.matmul(pT_ps[:], wp1_sb[:], oT1[:], start=True, stop=True)
            pT_sb = sb.tile([C, UP], F32)
            nc.vector.tensor_copy(out=pT_sb[:], in_=pT_ps[:])
            p_sb = sb.tile([P, JC, C], F32)
            for jc in range(JC):
                tp2 = ps_t.tile([P, C], F32, tag="t")
                nc.tensor.transpose(
                    out=tp2[:],
                    in_=pT_sb[:, jc * P : (jc + 1) * P],
                    identity=ident[0:C, 0:C],
                )
                nc.vector.tensor_copy(out=p_sb[:, jc, :], in_=tp2[:])

            # expand unique rows to all 4096 positions: out^T slice-by-slice
            # (lhsT = p chunks so only 4 small weight loads per slice), then
            # transpose each 128-col strip back to [n, c] and store
            SL = N // NS  # 512
            for ns in range(NS):
                obT = ps_ob.tile([C, SL], F32)
                for jc in range(JC):
                    nc.tensor.matmul(
                        obT[:],
                        p_sb[:, jc, :],
                        gt_sb[:, jc, ns * SL : (ns + 1) * SL],
                        start=(jc == 0),
                        stop=(jc == JC - 1),
                    )
                obT_sb = obt_sb_pool.tile([C, SL], F32)
                nc.vector.tensor_copy(out=obT_sb[:], in_=obT[:])
                for k in range(SL // P):
                    t = ns * (SL // P) + k
                    on_ps = ps_t.tile([P, C], F32, tag="t")
                    nc.tensor.transpose(
                        out=on_ps[:],
                        in_=obT_sb[:, k * P : (k + 1) * P],
                        identity=ident[0:C, 0:C],
                    )
                    o_sb = opool.tile([P, C], F32)
                    nc.vector.tensor_copy(out=o_sb[:], in_=on_ps[:])
                    nc.sync.dma_start(out=out[t * P : (t + 1) * P, :], in_=o_sb[:])

    nc.compile()
    return nc


_nc_cache = None


def _get_nc():
    global _nc_cache
    if _nc_cache is None:
        _nc_cache = build_nc()
    return _nc_cache


def make_in_maps(x, wq, bq, wk, bk, wv, bv, wp, bp):
    f = lambda a: np.ascontiguousarray(np.asarray(a, dtype=np.float32))
    x = f(x)
    shared = {
        "wq1": np.concatenate([f(wq).T, f(bq)[None, :]], 0),
        "wkn": f(wk),
        "wv1": np.concatenate([f(wv).T, f(bv)[None, :]], 0),
        "wp1": np.concatenate([f(wp).T, f(bp)[None, :]], 0),
        "wu": W_U,
        "gt": GT,
    }
    shared = {k: np.ascontiguousarray(v) for k, v in shared.items()}
    return [
        {"xb": np.ascontiguousarray(x[b].reshape(N, C)), **shared} for b in range(B)
    ]


def kernel_with_results(trace=False, **inputs):
    in_maps = make_in_maps(**inputs)
    nc = _get_nc()
    res = run_bass_kernel_spmd(nc, in_maps, core_ids=list(range(B)), trace=trace)
    out = np.stack([r["out"] for r in res.results], 0).reshape(B, H, W, C)
    return out, res


def kernel(**inputs):
    out, _ = kernel_with_results(**inputs)
    return out


# revision 8
# speedup vs baseline: 1.5937x; 1.2157x over previous
"""Trainium2 Bass kernel for nn_Attention_78048145703090 (sparse_attention).

Math: the reference's [N,N] attention is rank-1 structured. Every row n of the
logit matrix is w_n * s where s[m] = scale * (q_center . k_m) is one shared
score vector per sample and w_n = exp(1 - dist_n) > 0 depends only on the grid
distance of n from the center. Softmax rows therefore only depend on w_n, and
only U=457 distinct w_n values exist on the 64x64 grid. The kernel computes
the 457 unique softmax rows, projects them, and expands back to 4096 rows
with a one-hot gather matmul.

Contractions used:
  - s = xf @ (scale * wk^T q_c) (+ const): row-constant terms drop out of
    softmax, so bk never enters; s is computed by one fused DVE
    mul+reduce per chunk against x in natural layout.
  - num = E' @ V = (E' @ xf) @ wv^T + den * bv, so V is never materialized
    and x is consumed in natural [m, c] layout as the matmul stationary
    operand (no input transposes at all).

The two large matmuls (E-contraction and the one-hot expansion) run in bf16
(measured end-to-end error 3e-3 absmax-relative vs the f32 reference);
everything feeding the softmax scores stays f32.

Sharding: data-parallel over B=8 across the 8 cores (one sample per core);
each core holds the full 64x64 weights.
"""

import sys

sys.path.insert(0, "/opt/trn_rl_repo")

import numpy as np

import concourse.bacc as bacc
import concourse.mybir as mybir
import concourse.tile as tile
from concourse import masks


def _install_profile_hook():
    """This image's antenv lacks axon_hooks; reconstruct it so
    run_bass_kernel_spmd(trace=True) can capture NTFF profiles. No-op for
    normal (untraced) runs."""
    import types

    try:
        import antenv.axon_hooks  # noqa: F401

        return
    except ImportError:
        pass
    try:
        import antenv

        m = types.ModuleType("antenv.axon_hooks")
        state = {"hook": None}
        m.set_axon_ntff_profile_hook = lambda h: state.__setitem__("hook", h)
        m.get_axon_ntff_profile_hook = lambda: state["hook"]
        sys.modules["antenv.axon_hooks"] = m
        antenv.axon_hooks = m
        from trn_agent_boot.trn_boot import _ntff_profile_via_ctypes

        m.set_axon_ntff_profile_hook(
            _ntff_profile_via_ctypes("/opt/axon/libaxon_pjrt.so")
        )
    except Exception:
        pass


_install_profile_hook()

from concourse.bass_utils import run_bass_kernel_spmd

B, H, W, C = 8, 64, 64, 64
N = H * W  # 4096
P = 128
NCH = N // P  # 32
CENTER = (H // 2) * W + (W // 2)  # 2080
C_CH = CENTER // P  # chunk holding the center row
SCALE = float(C) ** -0.5
F32 = mybir.dt.float32
BF16 = mybir.dt.bfloat16
NS = 8  # output column slices for the gather (N / 512)

# ---- compile-time constants derived from the distance grid ----
_yy, _xx = np.mgrid[0:H, 0:W]
_d2 = ((_yy - H // 2) ** 2 + (_xx - W // 2) ** 2).reshape(-1)  # [N] int
_uniq_d2, _g = np.unique(_d2, return_inverse=True)
U = len(_uniq_d2)  # 457
UP = 512  # padded to 4 partition chunks
JC = UP // P  # 4
W_U = np.zeros((1, UP), np.float32)
W_U[0, :U] = np.exp(np.float32(1.0) - np.sqrt(_uniq_d2.astype(np.float32)))
# one-hot gather matrix (bf16, exact), packed [P, JC, N]
import ml_dtypes
import os

BF16_GATHER = os.environ.get("K_BF16_GATHER", "1") == "1"
GT_SWDGE = os.environ.get("K_GT_SWDGE", "1") == "1"
USE_TTR = os.environ.get("K_TTR", "0") == "1"
GT_NP = ml_dtypes.bfloat16 if BF16_GATHER else np.float32
GT = np.zeros((P, JC, N), GT_NP)
GT[_g % P, _g // P, np.arange(N)] = 1.0

# chunk DMA order: center chunk first so the q_c chain starts immediately
_CH_ORDER = [C_CH] + [i for i in range(NCH) if i != C_CH]


def build_nc():
    nc = bacc.Bacc("TRN2", target_bir_lowering=False, debug=False, num_devices=B)
    xb = nc.dram_tensor("xb", [N, C], F32, kind="ExternalInput")
    wq1 = nc.dram_tensor("wq1", [C + 1, C], F32, kind="ExternalInput")
    wkn = nc.dram_tensor("wkn", [C, C], F32, kind="ExternalInput")
    wv1 = nc.dram_tensor("wv1", [C + 1, C], F32, kind="ExternalInput")
    wp1 = nc.dram_tensor("wp1", [C + 1, C], F32, kind="ExternalInput")
    wu = nc.dram_tensor("wu", [1, UP], F32, kind="ExternalInput")
    GTDT = BF16 if BF16_GATHER else F32
    gt = nc.dram_tensor("gt", [P, JC, N], GTDT, kind="ExternalInput")
    out = nc.dram_tensor("out", [N, C], F32, kind="ExternalOutput")

    xv = xb.ap().rearrange("(i p) c -> p i c", p=P)

    with tile.TileContext(nc) as tc:
        with (
            tc.tile_pool(name="consts", bufs=1) as consts,
            tc.tile_pool(name="sb", bufs=1) as sb,
            tc.tile_pool(name="xu_pool", bufs=2) as xu_pool,
            tc.tile_pool(name="epool", bufs=4) as epool,
            tc.tile_pool(name="opool", bufs=4) as opool,
            tc.tile_pool(name="obt_sb_pool", bufs=2) as obt_sb_pool,
            tc.tile_pool(name="ps_t", bufs=2, space="PSUM") as ps_t,
            tc.tile_pool(name="ps_yt", bufs=1, space="PSUM") as ps_yt,
            tc.tile_pool(name="ps_small", bufs=2, space="PSUM") as ps_small,
            tc.tile_pool(name="ps_ob", bufs=3, space="PSUM") as ps_ob,
        ):
            ident = consts.tile([P, P], F32)
            masks.make_identity(nc, ident[:])
            identb = consts.tile([P, P], BF16)
            masks.make_identity(nc, identb[:])
            ones_row = consts.tile([1, P], F32)
            nc.vector.memset(ones_row[:], 1.0)

            # small weights on the HWDGE queue first
            wq1_sb = consts.tile([C + 1, C], F32)
            nc.sync.dma_start(out=wq1_sb[:], in_=wq1[:])
            wkn_sb = consts.tile([C, C], F32)
            nc.sync.dma_start(out=wkn_sb[:], in_=wkn[:])
            wv1_sb = consts.tile([C + 1, C], F32)
            nc.sync.dma_start(out=wv1_sb[:], in_=wv1[:])
            wp1_sb = consts.tile([C + 1, C], F32)
            nc.sync.dma_start(out=wp1_sb[:], in_=wp1[:])
            wu_sb = consts.tile([1, UP], F32)
            nc.sync.dma_start(out=wu_sb[:], in_=wu[:])

            # x chunks (f32 natural + bf16 copy with ones column), center first
            x1_sb = sb.tile([P, NCH, C], F32)
            x1b_sb = sb.tile([P, NCH, C + 1], BF16)
            nc.vector.memset(x1b_sb[:, :, C : C + 1], 1.0)
            for i in _CH_ORDER:
                nc.sync.dma_start(out=x1_sb[:, i, :], in_=xv[:, i, :])
                nc.gpsimd.tensor_copy(out=x1b_sb[:, i, 0:C], in_=x1_sb[:, i, :])

            # the big one-hot matrix arrives on the SWDGE queue, needed late
            gt_sb = consts.tile([P, JC, N], GTDT)
            if GT_SWDGE:
                nc.gpsimd.dma_start(out=gt_sb[:], in_=gt[:])
            else:
                nc.sync.dma_start(out=gt_sb[:], in_=gt[:])

            # q_center: transpose the center chunk, take the center column
            qcr_sb = sb.tile([C + 1, 1], F32)
            nc.vector.memset(qcr_sb[:], 1.0)
            xrow_ps = ps_small.tile([C, P], F32, tag="m")
            nc.tensor.transpose(
                out=xrow_ps[:], in_=x1_sb[:, C_CH, :], identity=ident[:]
            )
            nc.vector.tensor_copy(
                out=qcr_sb[0:C, :], in_=xrow_ps[:, CENTER % P : CENTER % P + 1]
            )
            qc_ps = ps_small.tile([C, 1], F32, tag="m")
            nc.tensor.matmul(qc_ps[:], wq1_sb[:], qcr_sb[:], start=True, stop=True)
            qc_sb = sb.tile([C, 1], F32)
            nc.vector.tensor_copy(out=qc_sb[:], in_=qc_ps[:])

            # u_row = scale * (q_c^T wk) as a row, broadcast across partitions
            ur_ps = ps_small.tile([1, C], F32, tag="m")
            nc.tensor.matmul(ur_ps[:], qc_sb[:], wkn_sb[:], start=True, stop=True)
            ur_sb = sb.tile([1, C], F32)
            nc.scalar.mul(out=ur_sb[:], in_=ur_ps[:], mul=SCALE)
            ubc_ps = ps_small.tile([P, C], F32, tag="m")
            nc.tensor.matmul(ubc_ps[:], ones_row[:], ur_sb[:], start=True, stop=True)
            ubc_sb = sb.tile([P, C], F32)
            nc.vector.tensor_copy(out=ubc_sb[:], in_=ubc_ps[:])

            # s[m] = x[m, :] . u  -- one fused mul+reduce per chunk
            s_col = sb.tile([P, NCH], F32)
            for i in range(NCH):
                xu = xu_pool.tile([P, C], F32)
                if USE_TTR:
                    nc.vector.tensor_tensor_reduce(
                        out=xu[:],
                        in0=x1_sb[:, i, :],
                        in1=ubc_sb[:],
                        scale=1.0,
                        scalar=0.0,
                        op0=mybir.AluOpType.mult,
                        op1=mybir.AluOpType.add,
                        accum_out=s_col[:, i : i + 1],
                    )
                else:
                    nc.vector.tensor_mul(xu[:], x1_sb[:, i, :], ubc_sb[:])
                    nc.vector.reduce_sum(
                        out=s_col[:, i : i + 1], in_=xu[:], axis=mybir.AxisListType.X
                    )

            # global max of s (w_n > 0 so row max = w_n * max(s))
            mx = sb.tile([P, 1], F32)
            nc.vector.reduce_max(out=mx[:], in_=s_col[:], axis=mybir.AxisListType.X)
            mxT = ps_small.tile([1, P], F32, tag="m")
            nc.tensor.transpose(out=mxT[:], in_=mx[:], identity=ident[:])
            mxT_sb = sb.tile([1, P], F32)
            nc.vector.tensor_copy(out=mxT_sb[:], in_=mxT[:])
            mg = sb.tile([1, 1], F32)
            nc.vector.reduce_max(out=mg[:], in_=mxT_sb[:], axis=mybir.AxisListType.X)
            mb_ps = ps_small.tile([P, 1], F32, tag="m")
            nc.tensor.matmul(mb_ps[:], ones_row[:], mg[:], start=True, stop=True)
            mb_sb = sb.tile([P, 1], F32)
            nc.vector.tensor_copy(out=mb_sb[:], in_=mb_ps[:])
            shm = sb.tile([P, NCH], F32)
            nc.vector.tensor_scalar_sub(shm[:], s_col[:], mb_sb[:])

            # unique weights broadcast across partitions
            wb_ps = ps_small.tile([P, UP], F32, tag="m")
            nc.tensor.matmul(wb_ps[:], ones_row[:], wu_sb[:], start=True, stop=True)
            wb_sb = sb.tile([P, UP], F32)
            nc.vector.tensor_copy(out=wb_sb[:], in_=wb_ps[:])

            # E'[m, j] = exp(sh[m] * w_u[j]) (bf16); accumulate YT = [x|1]^T E'
            # rows 0..63 = (E' @ xf)^T, row 64 = den
            yt_ps = ps_yt.tile([C + 1, UP], F32)
            for i in range(NCH):
                e_i = epool.tile([P, UP], BF16)
                nc.scalar.activation(
                    out=e_i[:],
                    in_=wb_sb[:],
                    func=mybir.ActivationFunctionType.Exp,
                    scale=shm[:, i : i + 1],
                )
                nc.tensor.matmul(
                    yt_ps[:],
                    x1b_sb[:, i, :],
                    e_i[:],
                    start=(i == 0),
                    stop=(i == NCH - 1),
                )

            ytd_sb = sb.tile([C + 1, UP], F32)
            nc.vector.tensor_copy(out=ytd_sb[:], in_=yt_ps[:])
            # num^T = [wv.T|bv]^T @ [Y|den]  (bias folds against the den row)
            numT_ps = ps_small.tile([C, UP], F32, tag="m")
            nc.tensor.matmul(numT_ps[:], wv1_sb[:], ytd_sb[:], start=True, stop=True)
            # r = 1/den broadcast across partitions
            r_sb = sb.tile([1, UP], F32)
            nc.vector.reciprocal(out=r_sb[:], in_=ytd_sb[C : C + 1, :])
            rb_ps = ps_small.tile([C, UP], F32, tag="m")
            nc.tensor.matmul(rb_ps[:], ones_row[:, 0:C], r_sb[:], start=True, stop=True)
            rb_sb = sb.tile([C, UP], F32)
            nc.vector.tensor_copy(out=rb_sb[:], in_=rb_ps[:])
            # o^T = num^T * r, with a ones row for the bias fold of wp
            oT1 = sb.tile([C + 1, UP], F32)
            nc.vector.memset(oT1[C : C + 1, :], 1.0)
            nc.vector.tensor_mul(oT1[0:C, :], numT_ps[:], rb_sb[:])

            # p^T = [wp.T|bp]^T @ oT1 -> [C, UP] (to bf16), transpose to chunks
            pT_ps = ps_small.tile([C, UP], F32, tag="m")
            nc.tensor.matmul(pT_ps[:], wp1_sb[:], oT1[:], start=True, stop=True)
            pT_sb = sb.tile([C, UP], GTDT)
            nc.vector.tensor_copy(out=pT_sb[:], in_=pT_ps[:])
            p_sb = sb.tile([P, JC, C], GTDT)
            for jc in range(JC):
                tp2 = ps_t.tile([P, C], GTDT, tag="tb")
                nc.tensor.transpose(
                    out=tp2[:],
                    in_=pT_sb[:, jc * P : (jc + 1) * P],
                    identity=(identb if BF16_GATHER else ident)[0:C, 0:C],
                )
                nc.vector.tensor_copy(out=p_sb[:, jc, :], in_=tp2[:])

            # expand unique rows to all 4096 positions: out^T slice-by-slice,
            # transpose each 128-col strip back to [n, c] (exact bf16 values),
            # convert to f32 on the final copy and store
            SL = N // NS  # 512
            for ns in range(NS):
                obT = ps_ob.tile([C, SL], F32)
                for jc in range(JC):
                    nc.tensor.matmul(
                        obT[:],
                        p_sb[:, jc, :],
                        gt_sb[:, jc, ns * SL : (ns + 1) * SL],
                        start=(jc == 0),
                        stop=(jc == JC - 1),
                    )
                obT_sb = obt_sb_pool.tile([C, SL], GTDT)
                nc.vector.tensor_copy(out=obT_sb[:], in_=obT[:])
                for k in range(SL // P):
                    t = ns * (SL // P) + k
                    on_ps = ps_t.tile([P, C], GTDT, tag="tb")
                    nc.tensor.transpose(
                        out=on_ps[:],
                        in_=obT_sb[:, k * P : (k + 1) * P],
                        identity=(identb if BF16_GATHER else ident)[0:C, 0:C],
                    )
                    o_sb = opool.tile([P, C], F32)
                    nc.vector.tensor_copy(out=o_sb[:], in_=on_ps[:])
                    nc.sync.dma_start(out=out[t * P : (t + 1) * P, :], in_=o_sb[:])

    nc.compile()
    return nc


_nc_cache = None


def _get_nc():
    global _nc_cache
    if _nc_cache is None:
        _nc_cache = build_nc()
    return _nc_cache


def make_in_maps(x, wq, bq, wk, bk, wv, bv, wp, bp):
    f = lambda a: np.ascontiguousarray(np.asarray(a, dtype=np.float32))
    x = f(x)
    shared = {
        "wq1": np.concatenate([f(wq).T, f(bq)[None, :]], 0),
        "wkn": f(wk),
        "wv1": np.concatenate([f(wv).T, f(bv)[None, :]], 0),
        "wp1": np.concatenate([f(wp).T, f(bp)[None, :]], 0),
        "wu": W_U,
        "gt": GT,
    }
    shared = {k: np.ascontiguousarray(v) for k, v in shared.items()}
    return [
        {"xb": np.ascontiguousarray(x[b].reshape(N, C)), **shared} for b in range(B)
    ]


def kernel_with_results(trace=False, **inputs):
    in_maps = make_in_maps(**inputs)
    nc = _get_nc()
    res = run_bass_kernel_spmd(nc, in_maps, core_ids=list(range(B)), trace=trace)
    out = np.stack([r["out"] for r in res.results], 0).reshape(B, H, W, C)
    return out, res


def kernel(**inputs):
    out, _ = kernel_with_results(**inputs)
    return out


# revision 9
# speedup vs baseline: 1.8972x; 1.1904x over previous
"""Trainium2 Bass kernel for nn_Attention_78048145703090 (sparse_attention).

Math: the reference's [N,N] attention is rank-1 structured. Every row n of the
logit matrix is w_n * s where s[m] = scale * (q_center . k_m) is one shared
score vector per sample and w_n = exp(1 - dist_n) > 0 depends only on the grid
distance of n from the center. Softmax rows therefore only depend on w_n, and
only U=457 distinct w_n values exist on the 64x64 grid. The kernel computes
the 457 unique softmax rows, projects them, and expands back to 4096 rows
with a one-hot gather matmul.

Contractions used:
  - s = xf @ (scale * wk^T q_c) (+ const): row-constant terms drop out of
    softmax, so bk never enters; s is computed by one fused DVE
    mul+reduce per chunk against x in natural layout.
  - num = E' @ V = (E' @ xf) @ wv^T + den * bv, so V is never materialized
    and x is consumed in natural [m, c] layout as the matmul stationary
    operand (no input transposes at all).

The two large matmuls (E-contraction and the one-hot expansion) run in bf16
(measured end-to-end error 3e-3 absmax-relative vs the f32 reference);
everything feeding the softmax scores stays f32.

Sharding: data-parallel over B=8 across the 8 cores (one sample per core);
each core holds the full 64x64 weights.
"""

import sys

sys.path.insert(0, "/opt/trn_rl_repo")

import numpy as np

import concourse.bacc as bacc
import concourse.mybir as mybir
import concourse.tile as tile
from concourse import masks


def _install_profile_hook():
    """This image's antenv lacks axon_hooks; reconstruct it so
    run_bass_kernel_spmd(trace=True) can capture NTFF profiles. No-op for
    normal (untraced) runs."""
    import types

    try:
        import antenv.axon_hooks  # noqa: F401

        return
    except ImportError:
        pass
    try:
        import antenv

        m = types.ModuleType("antenv.axon_hooks")
        state = {"hook": None}
        m.set_axon_ntff_profile_hook = lambda h: state.__setitem__("hook", h)
        m.get_axon_ntff_profile_hook = lambda: state["hook"]
        sys.modules["antenv.axon_hooks"] = m
        antenv.axon_hooks = m
        from trn_agent_boot.trn_boot import _ntff_profile_via_ctypes

        m.set_axon_ntff_profile_hook(
            _ntff_profile_via_ctypes("/opt/axon/libaxon_pjrt.so")
        )
    except Exception:
        pass


_install_profile_hook()

from concourse.bass_utils import run_bass_kernel_spmd

B, H, W, C = 8, 64, 64, 64
N = H * W  # 4096
P = 128
NCH = N // P  # 32
CENTER = (H // 2) * W + (W // 2)  # 2080
C_CH = CENTER // P  # chunk holding the center row
SCALE = float(C) ** -0.5
F32 = mybir.dt.float32
BF16 = mybir.dt.bfloat16
NS = 8  # output column slices for the gather (N / 512)

# ---- compile-time constants derived from the distance grid ----
_yy, _xx = np.mgrid[0:H, 0:W]
_d2 = ((_yy - H // 2) ** 2 + (_xx - W // 2) ** 2).reshape(-1)  # [N] int
_uniq_d2, _g = np.unique(_d2, return_inverse=True)
U = len(_uniq_d2)  # 457
UP = U  # no padding: exp/matmul streams only cover real uniques
JC = (U + P - 1) // P  # 4 chunks: 128,128,128,73
CS = [min(P, U - jc * P) for jc in range(JC)]
W_U = np.zeros((1, UP), np.float32)
W_U[0, :U] = np.exp(np.float32(1.0) - np.sqrt(_uniq_d2.astype(np.float32)))
# one-hot gather matrix (bf16, exact), packed [P, JC, N]
import ml_dtypes
import os

BF16_GATHER = os.environ.get("K_BF16_GATHER", "1") == "1"
GT_SWDGE = os.environ.get("K_GT_SWDGE", "1") == "1"
USE_TTR = os.environ.get("K_TTR", "0") == "1"
USE_STT = os.environ.get("K_STT", "1") == "1"
GT_NP = ml_dtypes.bfloat16 if BF16_GATHER else np.float32
GT = np.zeros((P, JC, N), GT_NP)
GT[_g % P, _g // P, np.arange(N)] = 1.0

# chunk-group DMA order: group holding the center chunk first
_GRP = 8  # chunks per x DMA
_G_ORDER = [C_CH // _GRP] + [g for g in range(NCH // _GRP) if g != C_CH // _GRP]
_CH_ORDER = [g * _GRP + k for g in _G_ORDER for k in range(_GRP)]


def build_nc():
    nc = bacc.Bacc("TRN2", target_bir_lowering=False, debug=False, num_devices=B)
    xb = nc.dram_tensor("xb", [N, C], F32, kind="ExternalInput")
    wq1 = nc.dram_tensor("wq1", [C + 1, C], F32, kind="ExternalInput")
    wkn = nc.dram_tensor("wkn", [C, C], F32, kind="ExternalInput")
    wv1 = nc.dram_tensor("wv1", [C + 1, C], F32, kind="ExternalInput")
    wp1 = nc.dram_tensor("wp1", [C + 1, C], F32, kind="ExternalInput")
    wu = nc.dram_tensor("wu", [1, UP], F32, kind="ExternalInput")
    GTDT = BF16 if BF16_GATHER else F32
    gt = nc.dram_tensor("gt", [P, JC, N], GTDT, kind="ExternalInput")
    out = nc.dram_tensor("out", [N, C], F32, kind="ExternalOutput")

    xv = xb.ap().rearrange("(i p) c -> p i c", p=P)

    with tile.TileContext(nc) as tc:
        with (
            tc.tile_pool(name="consts", bufs=1) as consts,
            tc.tile_pool(name="sb", bufs=1) as sb,
            tc.tile_pool(name="xu_pool", bufs=2) as xu_pool,
            tc.tile_pool(name="epool", bufs=4) as epool,
            tc.tile_pool(name="opool", bufs=4) as opool,
            tc.tile_pool(name="obt_sb_pool", bufs=2) as obt_sb_pool,
            tc.tile_pool(name="ps_t", bufs=2, space="PSUM") as ps_t,
            tc.tile_pool(name="ps_yt", bufs=1, space="PSUM") as ps_yt,
            tc.tile_pool(name="ps_small", bufs=2, space="PSUM") as ps_small,
            tc.tile_pool(name="ps_ob", bufs=3, space="PSUM") as ps_ob,
        ):
            ident = consts.tile([P, P], F32)
            masks.make_identity(nc, ident[:])
            identb = consts.tile([P, P], BF16)
            masks.make_identity(nc, identb[:])
            ones_row = consts.tile([1, P], F32)
            nc.vector.memset(ones_row[:], 1.0)

            # small weights on the HWDGE queue first
            wq1_sb = consts.tile([C + 1, C], F32)
            nc.sync.dma_start(out=wq1_sb[:], in_=wq1[:])
            wkn_sb = consts.tile([C, C], F32)
            nc.sync.dma_start(out=wkn_sb[:], in_=wkn[:])
            wv1_sb = consts.tile([C + 1, C], F32)
            nc.sync.dma_start(out=wv1_sb[:], in_=wv1[:])
            wp1_sb = consts.tile([C + 1, C], F32)
            nc.sync.dma_start(out=wp1_sb[:], in_=wp1[:])
            wu_sb = consts.tile([1, UP], F32)
            nc.sync.dma_start(out=wu_sb[:], in_=wu[:])

            # x chunks (f32 natural + bf16 copy with ones column), center first
            x1_sb = sb.tile([P, NCH, C], F32)
            x1b_sb = sb.tile([P, NCH, C + 1], BF16)
            nc.vector.memset(x1b_sb[:, :, C : C + 1], 1.0)
            for g in _G_ORDER:
                i0 = g * _GRP
                nc.sync.dma_start(
                    out=x1_sb[:, i0 : i0 + _GRP, :], in_=xv[:, i0 : i0 + _GRP, :]
                )
                for i in range(i0, i0 + _GRP):
                    nc.gpsimd.tensor_copy(
                        out=x1b_sb[:, i, 0:C], in_=x1_sb[:, i, :]
                    )

            # the big one-hot matrix arrives on the SWDGE queue, needed late
            gt_sb = consts.tile([P, JC, N], GTDT)
            if GT_SWDGE:
                nc.gpsimd.dma_start(out=gt_sb[:], in_=gt[:])
            else:
                nc.sync.dma_start(out=gt_sb[:], in_=gt[:])

            # q_center: transpose the center chunk, take the center column
            qcr_sb = sb.tile([C + 1, 1], F32)
            nc.vector.memset(qcr_sb[:], 1.0)
            xrow_ps = ps_small.tile([C, P], F32, tag="m")
            nc.tensor.transpose(
                out=xrow_ps[:], in_=x1_sb[:, C_CH, :], identity=ident[:]
            )
            nc.vector.tensor_copy(
                out=qcr_sb[0:C, :], in_=xrow_ps[:, CENTER % P : CENTER % P + 1]
            )
            qc_ps = ps_small.tile([C, 1], F32, tag="m")
            nc.tensor.matmul(qc_ps[:], wq1_sb[:], qcr_sb[:], start=True, stop=True)
            qc_sb = sb.tile([C, 1], F32)
            nc.vector.tensor_copy(out=qc_sb[:], in_=qc_ps[:])

            # u_row = scale * (q_c^T wk) as a row, broadcast across partitions
            ur_ps = ps_small.tile([1, C], F32, tag="m")
            nc.tensor.matmul(ur_ps[:], qc_sb[:], wkn_sb[:], start=True, stop=True)
            ur_sb = sb.tile([1, C], F32)
            nc.scalar.mul(out=ur_sb[:], in_=ur_ps[:], mul=SCALE)
            ubc_ps = ps_small.tile([P, C], F32, tag="m")
            nc.tensor.matmul(ubc_ps[:], ones_row[:], ur_sb[:], start=True, stop=True)
            ubc_sb = sb.tile([P, C], F32)
            nc.vector.tensor_copy(out=ubc_sb[:], in_=ubc_ps[:])

            # s[m] = x[m, :] . u  -- one fused mul+reduce per chunk
            s_col = sb.tile([P, NCH], F32)
            for i in range(NCH):
                xu = xu_pool.tile([P, C], F32)
                if USE_STT:
                    nc.vector.scalar_tensor_tensor(
                        out=xu[:],
                        in0=x1_sb[:, i, :],
                        scalar=1.0,
                        in1=ubc_sb[:],
                        op0=mybir.AluOpType.mult,
                        op1=mybir.AluOpType.mult,
                        accum_out=s_col[:, i : i + 1],
                    )
                elif USE_TTR:
                    nc.vector.tensor_tensor_reduce(
                        out=xu[:],
                        in0=x1_sb[:, i, :],
                        in1=ubc_sb[:],
                        scale=1.0,
                        scalar=0.0,
                        op0=mybir.AluOpType.mult,
                        op1=mybir.AluOpType.add,
                        accum_out=s_col[:, i : i + 1],
                    )
                else:
                    nc.vector.tensor_mul(xu[:], x1_sb[:, i, :], ubc_sb[:])
                    nc.vector.reduce_sum(
                        out=s_col[:, i : i + 1], in_=xu[:], axis=mybir.AxisListType.X
                    )

            # global max of s (w_n > 0 so row max = w_n * max(s))
            mx = sb.tile([P, 1], F32)
            nc.vector.reduce_max(out=mx[:], in_=s_col[:], axis=mybir.AxisListType.X)
            mxT = ps_small.tile([1, P], F32, tag="m")
            nc.tensor.transpose(out=mxT[:], in_=mx[:], identity=ident[:])
            mxT_sb = sb.tile([1, P], F32)
            nc.vector.tensor_copy(out=mxT_sb[:], in_=mxT[:])
            mg = sb.tile([1, 1], F32)
            nc.vector.reduce_max(out=mg[:], in_=mxT_sb[:], axis=mybir.AxisListType.X)
            mb_ps = ps_small.tile([P, 1], F32, tag="m")
            nc.tensor.matmul(mb_ps[:], ones_row[:], mg[:], start=True, stop=True)
            mb_sb = sb.tile([P, 1], F32)
            nc.vector.tensor_copy(out=mb_sb[:], in_=mb_ps[:])
            shm = sb.tile([P, NCH], F32)
            nc.vector.tensor_scalar_sub(shm[:], s_col[:], mb_sb[:])

            # unique weights broadcast across partitions
            wb_ps = ps_small.tile([P, UP], F32, tag="m")
            nc.tensor.matmul(wb_ps[:], ones_row[:], wu_sb[:], start=True, stop=True)
            wb_sb = sb.tile([P, UP], F32)
            nc.vector.tensor_copy(out=wb_sb[:], in_=wb_ps[:])

            # E'[m, j] = exp(sh[m] * w_u[j]) (bf16); accumulate YT = [x|1]^T E'
            # rows 0..63 = (E' @ xf)^T, row 64 = den
            yt_ps = ps_yt.tile([C + 1, UP], F32)
            for i in range(NCH):
                e_i = epool.tile([P, UP], BF16)
                nc.scalar.activation(
                    out=e_i[:],
                    in_=wb_sb[:],
                    func=mybir.ActivationFunctionType.Exp,
                    scale=shm[:, i : i + 1],
                )
                nc.tensor.matmul(
                    yt_ps[:],
                    x1b_sb[:, i, :],
                    e_i[:],
                    start=(i == 0),
                    stop=(i == NCH - 1),
                )

            ytd_sb = sb.tile([C + 1, UP], F32)
            nc.vector.tensor_copy(out=ytd_sb[:], in_=yt_ps[:])
            # num^T = [wv.T|bv]^T @ [Y|den]  (bias folds against the den row)
            numT_ps = ps_small.tile([C, UP], F32, tag="m")
            nc.tensor.matmul(numT_ps[:], wv1_sb[:], ytd_sb[:], start=True, stop=True)
            # r = 1/den broadcast across partitions
            r_sb = sb.tile([1, UP], F32)
            nc.vector.reciprocal(out=r_sb[:], in_=ytd_sb[C : C + 1, :])
            rb_ps = ps_small.tile([C, UP], F32, tag="m")
            nc.tensor.matmul(rb_ps[:], ones_row[:, 0:C], r_sb[:], start=True, stop=True)
            rb_sb = sb.tile([C, UP], F32)
            nc.vector.tensor_copy(out=rb_sb[:], in_=rb_ps[:])
            # o^T = num^T * r, with a ones row for the bias fold of wp
            oT1 = sb.tile([C + 1, UP], F32)
            nc.vector.memset(oT1[C : C + 1, :], 1.0)
            nc.vector.tensor_mul(oT1[0:C, :], numT_ps[:], rb_sb[:])

            # p^T = [wp.T|bp]^T @ oT1 -> [C, UP] (to bf16), transpose to chunks
            pT_ps = ps_small.tile([C, UP], F32, tag="m")
            nc.tensor.matmul(pT_ps[:], wp1_sb[:], oT1[:], start=True, stop=True)
            pT_sb = sb.tile([C, UP], GTDT)
            nc.vector.tensor_copy(out=pT_sb[:], in_=pT_ps[:])
            p_sb = sb.tile([P, JC, C], GTDT)
            for jc in range(JC):
                cs = CS[jc]
                tp2 = ps_t.tile([P, C], GTDT, tag="tb")
                nc.tensor.transpose(
                    out=tp2[0:cs, :],
                    in_=pT_sb[:, jc * P : jc * P + cs],
                    identity=(identb if BF16_GATHER else ident)[0:C, 0:C],
                )
                nc.vector.tensor_copy(out=p_sb[0:cs, jc, :], in_=tp2[0:cs, :])

            # expand unique rows to all 4096 positions: out^T slice-by-slice,
            # transpose each 128-col strip back to [n, c] (exact bf16 values),
            # convert to f32 on the final copy and store
            SL = N // NS  # 512
            ov = out.ap().rearrange("(s p) c -> p s c", p=P)  # [P, 32, C]
            for ns in range(NS):
                obT = ps_ob.tile([C, SL], F32)
                for jc in range(JC):
                    cs = CS[jc]
                    nc.tensor.matmul(
                        obT[:],
                        p_sb[0:cs, jc, :],
                        gt_sb[0:cs, jc, ns * SL : (ns + 1) * SL],
                        start=(jc == 0),
                        stop=(jc == JC - 1),
                    )
                obT_sb = obt_sb_pool.tile([C, SL], GTDT)
                nc.scalar.copy(out=obT_sb[:], in_=obT[:])
                o_sb = opool.tile([P, SL // P, C], F32)
                for k in range(SL // P):
                    on_ps = ps_t.tile([P, C], GTDT, tag="tb")
                    nc.tensor.transpose(
                        out=on_ps[:],
                        in_=obT_sb[:, k * P : (k + 1) * P],
                        identity=(identb if BF16_GATHER else ident)[0:C, 0:C],
                    )
                    nc.scalar.copy(out=o_sb[:, k, :], in_=on_ps[:])
                nc.sync.dma_start(
                    out=ov[:, ns * (SL // P) : (ns + 1) * (SL // P), :], in_=o_sb[:]
                )

    nc.compile()
    return nc


_nc_cache = None


def _get_nc():
    global _nc_cache
    if _nc_cache is None:
        _nc_cache = build_nc()
    return _nc_cache


def make_in_maps(x, wq, bq, wk, bk, wv, bv, wp, bp):
    f = lambda a: np.ascontiguousarray(np.asarray(a, dtype=np.float32))
    x = f(x)
    shared = {
        "wq1": np.concatenate([f(wq).T, f(bq)[None, :]], 0),
        "wkn": f(wk),
        "wv1": np.concatenate([f(wv).T, f(bv)[None, :]], 0),
        "wp1": np.concatenate([f(wp).T, f(bp)[None, :]], 0),
        "wu": W_U,
        "gt": GT,
    }
    shared = {k: np.ascontiguousarray(v) for k, v in shared.items()}
    return [
        {"xb": np.ascontiguousarray(x[b].reshape(N, C)), **shared} for b in range(B)
    ]


def kernel_with_results(trace=False, **inputs):
    in_maps = make_in_maps(**inputs)
    nc = _get_nc()
    res = run_bass_kernel_spmd(nc, in_maps, core_ids=list(range(B)), trace=trace)
    out = np.stack([r["out"] for r in res.results], 0).reshape(B, H, W, C)
    return out, res


def kernel(**inputs):
    out, _ = kernel_with_results(**inputs)
    return out


# revision 12
# speedup vs baseline: 1.9217x; 1.0129x over previous
"""Trainium2 Bass kernel for nn_Attention_78048145703090 (sparse_attention).

Math: the reference's [N,N] attention is rank-1 structured. Every row n of the
logit matrix is w_n * s where s[m] = scale * (q_center . k_m) is one shared
score vector per sample and w_n = exp(1 - dist_n) > 0 depends only on the grid
distance of n from the center. Softmax rows therefore only depend on w_n, and
only U=457 distinct w_n values exist on the 64x64 grid. The kernel computes
the 457 unique softmax rows, projects them, and expands back to 4096 rows
with a one-hot gather matmul.

Contractions used:
  - s = xf @ (scale * wk^T q_c) (+ const): row-constant terms drop out of
    softmax, so bk never enters; s is computed by one fused DVE
    mul+reduce per chunk against x in natural layout.
  - num = E' @ V = (E' @ xf) @ wv^T + den * bv, so V is never materialized
    and x is consumed in natural [m, c] layout as the matmul stationary
    operand (no input transposes at all).

The two large matmuls (E-contraction and the one-hot expansion) run in bf16
(measured end-to-end error 3e-3 absmax-relative vs the f32 reference);
everything feeding the softmax scores stays f32.

Sharding: data-parallel over B=8 across the 8 cores (one sample per core);
each core holds the full 64x64 weights.
"""

import sys

sys.path.insert(0, "/opt/trn_rl_repo")

import numpy as np

import concourse.bacc as bacc
import concourse.mybir as mybir
import concourse.tile as tile
from concourse import masks


def _install_profile_hook():
    """This image's antenv lacks axon_hooks; reconstruct it so
    run_bass_kernel_spmd(trace=True) can capture NTFF profiles. No-op for
    normal (untraced) runs."""
    import types

    try:
        import antenv.axon_hooks  # noqa: F401

        return
    except ImportError:
        pass
    try:
        import antenv

        m = types.ModuleType("antenv.axon_hooks")
        state = {"hook": None}
        m.set_axon_ntff_profile_hook = lambda h: state.__setitem__("hook", h)
        m.get_axon_ntff_profile_hook = lambda: state["hook"]
        sys.modules["antenv.axon_hooks"] = m
        antenv.axon_hooks = m
        from trn_agent_boot.trn_boot import _ntff_profile_via_ctypes

        m.set_axon_ntff_profile_hook(
            _ntff_profile_via_ctypes("/opt/axon/libaxon_pjrt.so")
        )
    except Exception:
        pass


_install_profile_hook()

from concourse.bass_utils import run_bass_kernel_spmd

B, H, W, C = 8, 64, 64, 64
N = H * W  # 4096
P = 128
NCH = N // P  # 32
CENTER = (H // 2) * W + (W // 2)  # 2080
C_CH = CENTER % NCH  # chunk (inner index) holding the center row: 0
C_PCOL = CENTER // NCH  # partition/column of the center row: 65
SCALE = float(C) ** -0.5
F32 = mybir.dt.float32
BF16 = mybir.dt.bfloat16
NS = 8  # output column slices for the gather (N / 512)

# ---- compile-time constants derived from the distance grid ----
_yy, _xx = np.mgrid[0:H, 0:W]
_d2 = ((_yy - H // 2) ** 2 + (_xx - W // 2) ** 2).reshape(-1)  # [N] int
_uniq_d2, _g = np.unique(_d2, return_inverse=True)
U = len(_uniq_d2)  # 457
UP = U  # no padding: exp/matmul streams only cover real uniques
JC = (U + P - 1) // P  # 4 chunks: 128,128,128,73
CS = [min(P, U - jc * P) for jc in range(JC)]
W_U = np.zeros((1, UP), np.float32)
W_U[0, :U] = np.exp(np.float32(1.0) - np.sqrt(_uniq_d2.astype(np.float32)))
# one-hot gather matrix (bf16, exact), packed [P, JC, N]
import ml_dtypes
import os

BF16_GATHER = os.environ.get("K_BF16_GATHER", "1") == "1"
GT_SWDGE = os.environ.get("K_GT_SWDGE", "1") == "1"
USE_TTR = os.environ.get("K_TTR", "0") == "1"
USE_STT = os.environ.get("K_STT", "1") == "1"
GT_NP = ml_dtypes.bfloat16 if BF16_GATHER else np.float32
GT = np.zeros((P, JC, N), GT_NP)
GT[_g % P, _g // P, np.arange(N)] = 1.0
# permute columns so each transposed 128-col strip is {p*32+s : p} for one s:
# after the final transposes the output sits in SBUF as [p, s, c] with
# row index n = p*32 + s, giving an 8KB-contiguous store per partition
GT = np.ascontiguousarray(
    GT.reshape(P, JC, P, NCH).transpose(0, 1, 3, 2).reshape(P, JC, N)
)




def build_nc():
    nc = bacc.Bacc("TRN2", target_bir_lowering=False, debug=False, num_devices=B)
    xb = nc.dram_tensor("xb", [N, C], F32, kind="ExternalInput")
    wq1 = nc.dram_tensor("wq1", [C + 1, C], F32, kind="ExternalInput")
    wkn = nc.dram_tensor("wkn", [C, C], F32, kind="ExternalInput")
    wv1 = nc.dram_tensor("wv1", [C + 1, C], F32, kind="ExternalInput")
    wp1 = nc.dram_tensor("wp1", [C + 1, C], F32, kind="ExternalInput")
    wu = nc.dram_tensor("wu", [1, UP], F32, kind="ExternalInput")
    GTDT = BF16 if BF16_GATHER else F32
    gt = nc.dram_tensor("gt", [P, JC, N], GTDT, kind="ExternalInput")
    out = nc.dram_tensor("out", [N, C], F32, kind="ExternalOutput")

    xv = xb.ap().rearrange("(p i) c -> p i c", p=P)

    with tile.TileContext(nc) as tc:
        with (
            tc.tile_pool(name="consts", bufs=1) as consts,
            tc.tile_pool(name="sb", bufs=1) as sb,
            tc.tile_pool(name="xu_pool", bufs=2) as xu_pool,
            tc.tile_pool(name="epool", bufs=4) as epool,
            tc.tile_pool(name="opool", bufs=4) as opool,
            tc.tile_pool(name="obt_sb_pool", bufs=2) as obt_sb_pool,
            tc.tile_pool(name="ps_t", bufs=2, space="PSUM") as ps_t,
            tc.tile_pool(name="ps_yt", bufs=1, space="PSUM") as ps_yt,
            tc.tile_pool(name="ps_small", bufs=2, space="PSUM") as ps_small,
            tc.tile_pool(name="ps_ob", bufs=3, space="PSUM") as ps_ob,
        ):
            ident = consts.tile([P, P], F32)
            masks.make_identity(nc, ident[:])
            identb = consts.tile([P, P], BF16)
            masks.make_identity(nc, identb[:])
            ones_row = consts.tile([1, P], F32)
            nc.vector.memset(ones_row[:], 1.0)

            # small weights on the HWDGE queue first
            wq1_sb = consts.tile([C + 1, C], F32)
            nc.sync.dma_start(out=wq1_sb[:], in_=wq1[:])
            wkn_sb = consts.tile([C, C], F32)
            nc.sync.dma_start(out=wkn_sb[:], in_=wkn[:])
            wv1_sb = consts.tile([C + 1, C], F32)
            nc.sync.dma_start(out=wv1_sb[:], in_=wv1[:])
            wp1_sb = consts.tile([C + 1, C], F32)
            nc.sync.dma_start(out=wp1_sb[:], in_=wp1[:])
            wu_sb = consts.tile([1, UP], F32)
            nc.sync.dma_start(out=wu_sb[:], in_=wu[:])

            # x chunks (f32 natural + bf16 copy with ones column), center first
            x1_sb = sb.tile([P, NCH, C], F32)
            x1b_sb = sb.tile([P, NCH, C + 1], BF16)
            nc.vector.memset(x1b_sb[:, :, C : C + 1], 1.0)
            nc.sync.dma_start(out=x1_sb[:], in_=xv[:])
            for i in range(NCH):
                nc.gpsimd.tensor_copy(out=x1b_sb[:, i, 0:C], in_=x1_sb[:, i, :])

            # the big one-hot matrix arrives on the SWDGE queue, needed late
            gt_sb = consts.tile([P, JC, N], GTDT)
            if GT_SWDGE:
                nc.gpsimd.dma_start(out=gt_sb[:], in_=gt[:])
            else:
                nc.sync.dma_start(out=gt_sb[:], in_=gt[:])

            # q_center: transpose the center chunk, take the center column
            qcr_sb = sb.tile([C + 1, 1], F32)
            nc.vector.memset(qcr_sb[:], 1.0)
            xrow_ps = ps_small.tile([C, P], F32, tag="m")
            nc.tensor.transpose(
                out=xrow_ps[:], in_=x1_sb[:, C_CH, :], identity=ident[:]
            )
            nc.vector.tensor_copy(
                out=qcr_sb[0:C, :], in_=xrow_ps[:, C_PCOL : C_PCOL + 1]
            )
            qc_ps = ps_small.tile([C, 1], F32, tag="m")
            nc.tensor.matmul(qc_ps[:], wq1_sb[:], qcr_sb[:], start=True, stop=True)
            qc_sb = sb.tile([C, 1], F32)
            nc.vector.tensor_copy(out=qc_sb[:], in_=qc_ps[:])

            # u_row = scale * (q_c^T wk) as a row, broadcast across partitions
            ur_ps = ps_small.tile([1, C], F32, tag="m")
            nc.tensor.matmul(ur_ps[:], qc_sb[:], wkn_sb[:], start=True, stop=True)
            ur_sb = sb.tile([1, C], F32)
            nc.scalar.mul(out=ur_sb[:], in_=ur_ps[:], mul=SCALE)
            ubc_ps = ps_small.tile([P, C], F32, tag="m")
            nc.tensor.matmul(ubc_ps[:], ones_row[:], ur_sb[:], start=True, stop=True)
            ubc_sb = sb.tile([P, C], F32)
            nc.vector.tensor_copy(out=ubc_sb[:], in_=ubc_ps[:])

            # s[m] = x[m, :] . u  -- one fused mul+reduce per chunk
            s_col = sb.tile([P, NCH], F32)
            for i in range(NCH):
                xu = xu_pool.tile([P, C], F32)
                if USE_STT:
                    nc.vector.scalar_tensor_tensor(
                        out=xu[:],
                        in0=x1_sb[:, i, :],
                        scalar=1.0,
                        in1=ubc_sb[:],
                        op0=mybir.AluOpType.mult,
                        op1=mybir.AluOpType.mult,
                        accum_out=s_col[:, i : i + 1],
                    )
                elif USE_TTR:
                    nc.vector.tensor_tensor_reduce(
                        out=xu[:],
                        in0=x1_sb[:, i, :],
                        in1=ubc_sb[:],
                        scale=1.0,
                        scalar=0.0,
                        op0=mybir.AluOpType.mult,
                        op1=mybir.AluOpType.add,
                        accum_out=s_col[:, i : i + 1],
                    )
                else:
                    nc.vector.tensor_mul(xu[:], x1_sb[:, i, :], ubc_sb[:])
                    nc.vector.reduce_sum(
                        out=s_col[:, i : i + 1], in_=xu[:], axis=mybir.AxisListType.X
                    )

            # global max of s (w_n > 0 so row max = w_n * max(s))
            mx = sb.tile([P, 1], F32)
            nc.vector.reduce_max(out=mx[:], in_=s_col[:], axis=mybir.AxisListType.X)
            mxT = ps_small.tile([1, P], F32, tag="m")
            nc.tensor.transpose(out=mxT[:], in_=mx[:], identity=ident[:])
            mxT_sb = sb.tile([1, P], F32)
            nc.vector.tensor_copy(out=mxT_sb[:], in_=mxT[:])
            mg = sb.tile([1, 1], F32)
            nc.vector.reduce_max(out=mg[:], in_=mxT_sb[:], axis=mybir.AxisListType.X)
            mb_ps = ps_small.tile([P, 1], F32, tag="m")
            nc.tensor.matmul(mb_ps[:], ones_row[:], mg[:], start=True, stop=True)
            mb_sb = sb.tile([P, 1], F32)
            nc.vector.tensor_copy(out=mb_sb[:], in_=mb_ps[:])
            shm = sb.tile([P, NCH], F32)
            nc.vector.tensor_scalar_sub(shm[:], s_col[:], mb_sb[:])

            # unique weights broadcast across partitions
            wb_ps = ps_small.tile([P, UP], F32, tag="m")
            nc.tensor.matmul(wb_ps[:], ones_row[:], wu_sb[:], start=True, stop=True)
            wb_sb = sb.tile([P, UP], F32)
            nc.vector.tensor_copy(out=wb_sb[:], in_=wb_ps[:])

            # E'[m, j] = exp(sh[m] * w_u[j]) (bf16); accumulate YT = [x|1]^T E'
            # rows 0..63 = (E' @ xf)^T, row 64 = den
            yt_ps = ps_yt.tile([C + 1, UP], F32)
            for i in range(NCH):
                e_i = epool.tile([P, UP], BF16)
                nc.scalar.activation(
                    out=e_i[:],
                    in_=wb_sb[:],
                    func=mybir.ActivationFunctionType.Exp,
                    scale=shm[:, i : i + 1],
                )
                nc.tensor.matmul(
                    yt_ps[:],
                    x1b_sb[:, i, :],
                    e_i[:],
                    start=(i == 0),
                    stop=(i == NCH - 1),
                )

            ytd_sb = sb.tile([C + 1, UP], F32)
            nc.vector.tensor_copy(out=ytd_sb[:], in_=yt_ps[:])
            # num^T = [wv.T|bv]^T @ [Y|den]  (bias folds against the den row)
            numT_ps = ps_small.tile([C, UP], F32, tag="m")
            nc.tensor.matmul(numT_ps[:], wv1_sb[:], ytd_sb[:], start=True, stop=True)
            # r = 1/den broadcast across partitions, then o^T = num^T * r
            r_sb = sb.tile([1, UP], F32)
            nc.vector.reciprocal(out=r_sb[:], in_=ytd_sb[C : C + 1, :])
            rb_ps = ps_small.tile([C, UP], F32, tag="m")
            nc.tensor.matmul(rb_ps[:], ones_row[:, 0:C], r_sb[:], start=True, stop=True)
            rb_sb = sb.tile([C, UP], F32)
            nc.vector.tensor_copy(out=rb_sb[:], in_=rb_ps[:])
            oT1 = sb.tile([C + 1, UP], F32)
            nc.vector.memset(oT1[C : C + 1, :], 1.0)
            nc.vector.tensor_mul(oT1[0:C, :], numT_ps[:], rb_sb[:])

            # p^T = [wp.T|bp]^T @ oT1 -> [C, UP] (to bf16), transpose to chunks
            pT_ps = ps_small.tile([C, UP], F32, tag="m")
            nc.tensor.matmul(pT_ps[:], wp1_sb[:], oT1[:], start=True, stop=True)
            pT_sb = sb.tile([C, UP], GTDT)
            nc.vector.tensor_copy(out=pT_sb[:], in_=pT_ps[:])
            p_sb = sb.tile([P, JC, C], GTDT)
            for jc in range(JC):
                cs = CS[jc]
                tp2 = ps_t.tile([P, C], GTDT, tag="tb")
                nc.tensor.transpose(
                    out=tp2[0:cs, :],
                    in_=pT_sb[:, jc * P : jc * P + cs],
                    identity=(identb if BF16_GATHER else ident)[0:C, 0:C],
                )
                nc.vector.tensor_copy(out=p_sb[0:cs, jc, :], in_=tp2[0:cs, :])

            # expand unique rows to all 4096 positions: out^T slice-by-slice,
            # transpose each 128-col strip back to [n, c] (exact bf16 values),
            # convert to f32 on the final copy and store
            SL = N // NS  # 512 permuted columns = 4 s-slots per slice
            SK = SL // P  # 4
            ov = out.ap().rearrange("(p s) c -> p s c", p=P)  # [P, 32, C]
            o_big = sb.tile([P, NCH, C], F32)
            for ns in range(NS):
                obT = ps_ob.tile([C, SL], F32)
                for jc in range(JC):
                    cs = CS[jc]
                    nc.tensor.matmul(
                        obT[:],
                        p_sb[0:cs, jc, :],
                        gt_sb[0:cs, jc, ns * SL : (ns + 1) * SL],
                        start=(jc == 0),
                        stop=(jc == JC - 1),
                    )
                obT_sb = obt_sb_pool.tile([C, SL], GTDT)
                nc.vector.tensor_copy(out=obT_sb[:], in_=obT[:])
                for k in range(SK):
                    s_slot = ns * SK + k
                    on_ps = ps_t.tile([P, C], GTDT, tag="tb")
                    nc.tensor.transpose(
                        out=on_ps[:],
                        in_=obT_sb[:, k * P : (k + 1) * P],
                        identity=(identb if BF16_GATHER else ident)[0:C, 0:C],
                    )
                    nc.vector.tensor_copy(out=o_big[:, s_slot, :], in_=on_ps[:])
                if ns % 2 == 1:
                    s0 = (ns - 1) * SK
                    nc.sync.dma_start(
                        out=ov[:, s0 : s0 + 2 * SK, :], in_=o_big[:, s0 : s0 + 2 * SK, :]
                    )

    nc.compile()
    return nc


_nc_cache = None


def _get_nc():
    global _nc_cache
    if _nc_cache is None:
        _nc_cache = build_nc()
    return _nc_cache


def make_in_maps(x, wq, bq, wk, bk, wv, bv, wp, bp):
    f = lambda a: np.ascontiguousarray(np.asarray(a, dtype=np.float32))
    x = f(x)
    shared = {
        "wq1": np.concatenate([f(wq).T, f(bq)[None, :]], 0),
        "wkn": f(wk),
        "wv1": np.concatenate([f(wv).T, f(bv)[None, :]], 0),
        "wp1": np.concatenate([f(wp).T, f(bp)[None, :]], 0),
        "wu": W_U,
        "gt": GT,
    }
    shared = {k: np.ascontiguousarray(v) for k, v in shared.items()}
    return [
        {"xb": np.ascontiguousarray(x[b].reshape(N, C)), **shared} for b in range(B)
    ]


def kernel_with_results(trace=False, **inputs):
    in_maps = make_in_maps(**inputs)
    nc = _get_nc()
    res = run_bass_kernel_spmd(nc, in_maps, core_ids=list(range(B)), trace=trace)
    out = np.stack([r["out"] for r in res.results], 0).reshape(B, H, W, C)
    return out, res


def kernel(**inputs):
    out, _ = kernel_with_results(**inputs)
    return out
